# revision 4
# baseline (speedup 1.0000x reference)
"""Lfm2ShortConv decode-step kernel for 8x TRN2 NeuronCores (Bass/Tile).

Sharding: tensor-parallel over the channel dim H=2048 -> 256 channels/core.
 - in_proj: each core computes proj[:, ch] for its 3x256 W_in rows (weights
   read exactly once across cores).
 - gated depthwise conv + state gather/scatter: channelwise independent; each
   core owns a contiguous [8192, 256, 2] shard of the conv-state pool,
   bulk-copies it input->output (DRAM->DRAM) and indirect-scatters the 256
   updated rows.
 - out_proj: contraction over H is split; each core emits a partial [256,2048]
   output, summed on host (the unshard step).
"""

import numpy as np
from contextlib import ExitStack

B = 256          # batch / requests
H = 2048         # hidden channels
HC = H // 8      # channels per core = 256
POOL = 8192      # request pool slots
P = 128          # partitions
KT = H // P      # contraction tiles for in_proj = 16
N_CORES = 8

_CACHE = {}


def _build():
    import concourse.bass as bass
    import concourse.tile as tile
    import concourse.mybir as mybir
    from concourse import bacc
    from concourse.masks import make_identity

    f32 = mybir.dt.float32
    i32 = mybir.dt.int32

    nc = bacc.Bacc("TRN2", target_bir_lowering=False, debug=False)

    hs_t = nc.dram_tensor("hs_t", [H, B], f32, kind="ExternalInput").ap()
    w_in_t = nc.dram_tensor("w_in_t", [H, 3 * HC], f32, kind="ExternalInput").ap()
    w_out_t = nc.dram_tensor("w_out_t", [HC, H], f32, kind="ExternalInput").ap()
    w_conv = nc.dram_tensor("w_conv", [P, 3 * HC], f32, kind="ExternalInput").ap()
    state_in = nc.dram_tensor("state_in", [POOL, 2 * HC], f32, kind="ExternalInput").ap()
    idx_d = nc.dram_tensor("idx", [B, 1], i32, kind="ExternalInput").ap()

    out_p = nc.dram_tensor("out_p", [B, H], f32, kind="ExternalOutput").ap()
    state_out = nc.dram_tensor("state_out", [POOL, 2 * HC], f32, kind="ExternalOutput").ap()

    MT = B // P  # 2 request tiles

    with tile.TileContext(nc) as tc:
        with ExitStack() as ctx:
            const = ctx.enter_context(tc.tile_pool(name="const", bufs=1))
            wstream = ctx.enter_context(tc.tile_pool(name="wstream", bufs=3))
            sb = ctx.enter_context(tc.tile_pool(name="sb", bufs=1))
            osb_pool = ctx.enter_context(tc.tile_pool(name="osb", bufs=3))
            psum = ctx.enter_context(tc.tile_pool(name="psum", bufs=1, space="PSUM"))
            psum2 = ctx.enter_context(tc.tile_pool(name="psum2", bufs=2, space="PSUM"))

            # ---- bulk copy of the state pool shard, DRAM->DRAM on ACT ring ----
            NCHUNK = 8
            rows = POOL // NCHUNK
            for ci in range(NCHUNK):
                nc.scalar.dma_start(
                    out=state_out[ci * rows:(ci + 1) * rows, :],
                    in_=state_in[ci * rows:(ci + 1) * rows, :],
                )

            # ---- constants ----
            wconv_sb = const.tile([P, 3 * HC], f32, tag="wconv", name="wconv")
            nc.sync.dma_start(out=wconv_sb[:], in_=w_conv[:])
            ident = const.tile([P, P], f32, tag="ident", name="ident")
            make_identity(nc, ident[:])
            idx_sb = []
            for m in range(MT):
                t = const.tile([P, 1], i32, tag=f"idx{m}", name=f"idx{m}")
                nc.sync.dma_start(out=t[:], in_=idx_d[m * P:(m + 1) * P, :])
                idx_sb.append(t)
            wout_sb = []
            for c in range(HC // P):
                t = const.tile([P, H], f32, tag=f"wout{c}", name=f"wout{c}")
                nc.sync.dma_start(out=t[:], in_=w_out_t[c * P:(c + 1) * P, :])
                wout_sb.append(t)

            # ---- in_proj: proj[B, 768] = hs.T^T @ w_in_t, K=2048 streamed ----
            # psum layout per m-tile: pa = proj[:, 0:512] (B_gate | C_gate),
            #                         pb = proj[:, 512:768] (x)
            pa = [psum.tile([P, 512], f32, tag=f"pa{m}", name=f"pa{m}") for m in range(MT)]
            pb = [psum.tile([P, 256], f32, tag=f"pb{m}", name=f"pb{m}") for m in range(MT)]
            for k in range(KT):
                hs_k = wstream.tile([P, B], f32, tag="hsk", name="hsk")
                nc.sync.dma_start(out=hs_k[:], in_=hs_t[k * P:(k + 1) * P, :])
                w_k = wstream.tile([P, 3 * HC], f32, tag="wk", name="wk")
                nc.sync.dma_start(out=w_k[:], in_=w_in_t[k * P:(k + 1) * P, :])
                for m in range(MT):
                    lhsT = hs_k[:, m * P:(m + 1) * P]
                    nc.tensor.matmul(
                        pa[m][:], lhsT, w_k[:, 0:512],
                        start=(k == 0), stop=(k == KT - 1),
                    )
                    nc.tensor.matmul(
                        pb[m][:], lhsT, w_k[:, 512:768],
                        start=(k == 0), stop=(k == KT - 1),
                    )

            # ---- gather current conv state rows for each request ----
            cur = []
            for m in range(MT):
                t = sb.tile([P, 2 * HC], f32, tag=f"cur{m}", name=f"cur{m}")
                nc.gpsimd.indirect_dma_start(
                    out=t[:],
                    out_offset=None,
                    in_=state_in[:],
                    in_offset=bass.IndirectOffsetOnAxis(ap=idx_sb[m][:, :1], axis=0),
                )
                cur.append(t)

            # ---- gating + depthwise conv (channelwise; request-partition) ----
            y_sb = []
            upd = []
            for m in range(MT):
                x_sb = sb.tile([P, HC], f32, tag=f"x{m}", name=f"x{m}")
                nc.vector.tensor_copy(out=x_sb[:], in_=pb[m][:])
                bx = sb.tile([P, HC], f32, tag=f"bx{m}", name=f"bx{m}")
                nc.vector.tensor_mul(out=bx[:], in0=pa[m][:, 0:HC], in1=x_sb[:])

                cur_k = cur[m][:].rearrange("p (c k) -> p c k", k=2)
                # conv_out = cur0*w0 + cur1*w1 + bx*w2
                t0 = sb.tile([P, HC], f32, tag=f"t0{m}", name=f"t0{m}")
                nc.vector.tensor_mul(out=t0[:], in0=cur_k[:, :, 0], in1=wconv_sb[:, 0:HC])
                t1 = sb.tile([P, HC], f32, tag=f"t1{m}", name=f"t1{m}")
                nc.vector.tensor_mul(out=t1[:], in0=cur_k[:, :, 1], in1=wconv_sb[:, HC:2 * HC])
                t2 = sb.tile([P, HC], f32, tag=f"t2{m}", name=f"t2{m}")
                nc.vector.tensor_mul(out=t2[:], in0=bx[:], in1=wconv_sb[:, 2 * HC:3 * HC])
                nc.vector.tensor_add(out=t0[:], in0=t0[:], in1=t1[:])
                nc.vector.tensor_add(out=t0[:], in0=t0[:], in1=t2[:])
                # y = C_gate * conv_out
                y = sb.tile([P, HC], f32, tag=f"y{m}", name=f"y{m}")
                nc.vector.tensor_mul(out=y[:], in0=pa[m][:, HC:2 * HC], in1=t0[:])
                y_sb.append(y)

                # updated state rows: [cur[:,:,1], bx] interleaved as (c, k)
                u = sb.tile([P, 2 * HC], f32, tag=f"upd{m}", name=f"upd{m}")
                u_k = u[:].rearrange("p (c k) -> p c k", k=2)
                nc.vector.tensor_copy(out=u_k[:, :, 0], in_=cur_k[:, :, 1])
                nc.vector.tensor_copy(out=u_k[:, :, 1], in_=bx[:])
                upd.append(u)

            # ---- scatter updated rows into the output state pool ----
            for m in range(MT):
                nc.gpsimd.indirect_dma_start(
                    out=state_out[:],
                    out_offset=bass.IndirectOffsetOnAxis(ap=idx_sb[m][:, :1], axis=0),
                    in_=upd[m][:],
                    in_offset=None,
                )

            # ---- transpose y to channel-partition layout for out_proj ----
            yt_sb = []
            for c in range(HC // P):
                pt = psum2.tile([P, B], f32, tag="pt", name="pt")
                for m in range(MT):
                    nc.tensor.transpose(
                        out=pt[:, m * P:(m + 1) * P],
                        in_=y_sb[m][:, c * P:(c + 1) * P],
                        identity=ident[:],
                    )
                t = sb.tile([P, B], f32, tag=f"yt{c}", name=f"yt{c}")
                nc.vector.tensor_copy(out=t[:], in_=pt[:])
                yt_sb.append(t)

            # ---- out_proj partial: out[B, 2048] = y^T.T @ w_out_t ----
            NO = H // 512  # 4 chunks of 512
            for m in range(MT):
                for n in range(NO):
                    po = psum2.tile([P, 512], f32, tag="po", name="po")
                    for c in range(HC // P):
                        nc.tensor.matmul(
                            po[:],
                            yt_sb[c][:, m * P:(m + 1) * P],
                            wout_sb[c][:, n * 512:(n + 1) * 512],
                            start=(c == 0), stop=(c == HC // P - 1),
                        )
                    ot = osb_pool.tile([P, 512], f32, tag="osb", name="osb")
                    nc.vector.tensor_copy(out=ot[:], in_=po[:])
                    nc.sync.dma_start(
                        out=out_p[m * P:(m + 1) * P, n * 512:(n + 1) * 512],
                        in_=ot[:],
                    )

    nc.compile()
    return nc


def _get_nc():
    if "nc" not in _CACHE:
        _CACHE["nc"] = _build()
    return _CACHE["nc"]


def make_in_maps(hidden_states, conv_state, req_pool_indices, W_in, W_out, conv_w):
    hs_t = np.ascontiguousarray(hidden_states.T.astype(np.float32, copy=False))
    idx = np.ascontiguousarray(req_pool_indices.astype(np.int32).reshape(B, 1))
    in_maps = []
    for c in range(N_CORES):
        lo, hi = c * HC, (c + 1) * HC
        w_in_rows = np.concatenate(
            [W_in[lo:hi, :], W_in[H + lo:H + hi, :], W_in[2 * H + lo:2 * H + hi, :]],
            axis=0,
        )  # [768, 2048]
        w_in_t = np.ascontiguousarray(w_in_rows.T)  # [2048, 768]
        w_out_t = np.ascontiguousarray(W_out[:, lo:hi].T)  # [256, 2048]
        wc = conv_w[lo:hi, :]  # [256, 3]
        wline = np.concatenate([wc[:, 0], wc[:, 1], wc[:, 2]])  # [768]
        w_conv_b = np.ascontiguousarray(np.broadcast_to(wline[None, :], (P, 3 * HC)))
        state_c = np.ascontiguousarray(conv_state[:, lo:hi, :]).reshape(POOL, 2 * HC)
        in_maps.append({
            "hs_t": hs_t,
            "w_in_t": w_in_t.astype(np.float32, copy=False),
            "w_out_t": w_out_t.astype(np.float32, copy=False),
            "w_conv": w_conv_b.astype(np.float32, copy=False),
            "state_in": state_c.astype(np.float32, copy=False),
            "idx": idx,
        })
    return in_maps


def unshard(results):
    out = np.zeros((B, H), dtype=np.float64)
    for r in results:
        out += r["out_p"].astype(np.float64)
    out = out.astype(np.float32)
    state = np.stack(
        [r["state_out"].reshape(POOL, HC, 2) for r in results], axis=1
    ).reshape(POOL, H, 2)
    return out, state


def run_sharded(inputs, trace=False, trace_cores=None):
    from concourse import bass_utils
    nc = _get_nc()
    in_maps = make_in_maps(**inputs)
    br = bass_utils.run_bass_kernel_spmd(
        nc, in_maps, list(range(N_CORES)), trace=trace, trace_cores=trace_cores,
    )
    return br


def kernel(hidden_states, conv_state, req_pool_indices, W_in, W_out, conv_w):
    br = run_sharded(dict(
        hidden_states=hidden_states, conv_state=conv_state,
        req_pool_indices=req_pool_indices, W_in=W_in, W_out=W_out, conv_w=conv_w,
    ))
    return unshard(br.results)


# revision 7
# speedup vs baseline: 5.8376x; 5.8376x over previous
"""Lfm2ShortConv decode-step kernel for 8x TRN2 NeuronCores (Bass/Tile).

Sharding: tensor-parallel over the channel dim H=2048 -> 256 channels/core.
 - in_proj: each core computes proj[:, ch] for its 3x256 W_in rows (weights
   read exactly once across cores).
 - gated depthwise conv + state gather/scatter: channelwise independent; each
   core owns a contiguous [8192, 256, 2] shard of the conv-state pool,
   bulk-copies it input->output (DRAM->DRAM) and indirect-scatters the 256
   updated rows.
 - out_proj: contraction over H is split; each core emits a partial [256,2048]
   output, summed on host (the unshard step).
"""

import numpy as np
from contextlib import ExitStack

B = 256          # batch / requests
H = 2048         # hidden channels
HC = H // 8      # channels per core = 256
POOL = 8192      # request pool slots
P = 128          # partitions
KT = H // P      # contraction tiles for in_proj = 16
N_CORES = 8

_CACHE = {}


def _build(repeats=1, do_copy=True, do_inproj=True, do_conv=True, do_outproj=True):
    import concourse.bass as bass
    import concourse.tile as tile
    import concourse.mybir as mybir
    from concourse import bacc
    from concourse.masks import make_identity

    f32 = mybir.dt.float32
    i32 = mybir.dt.int32

    nc = bacc.Bacc("TRN2", target_bir_lowering=False, debug=False)

    hs_t = nc.dram_tensor("hs_t", [H, B], f32, kind="ExternalInput").ap()
    w_in_t = nc.dram_tensor("w_in_t", [H, 3 * HC], f32, kind="ExternalInput").ap()
    w_out_t = nc.dram_tensor("w_out_t", [HC, H], f32, kind="ExternalInput").ap()
    w_conv = nc.dram_tensor("w_conv", [P, 3 * HC], f32, kind="ExternalInput").ap()
    state_in = nc.dram_tensor("state_in", [POOL, 2 * HC], f32, kind="ExternalInput").ap()
    idx_d = nc.dram_tensor("idx", [B, 1], i32, kind="ExternalInput").ap()

    out_p = nc.dram_tensor("out_p", [B, H], f32, kind="ExternalOutput").ap()
    state_out = nc.dram_tensor("state_out", [POOL, 2 * HC], f32, kind="ExternalOutput").ap()

    MT = B // P  # 2 request tiles

    with tile.TileContext(nc) as tc:
        with ExitStack() as ctx:
            const = ctx.enter_context(tc.tile_pool(name="const", bufs=1))
            wstream = ctx.enter_context(tc.tile_pool(name="wstream", bufs=3))
            sb = ctx.enter_context(tc.tile_pool(name="sb", bufs=1))
            osb_pool = ctx.enter_context(tc.tile_pool(name="osb", bufs=3))
            psum = ctx.enter_context(tc.tile_pool(name="psum", bufs=1, space="PSUM"))
            psum2 = ctx.enter_context(tc.tile_pool(name="psum2", bufs=2, space="PSUM"))

            # ---- constants (loaded once) ----
            wconv_sb = const.tile([P, 3 * HC], f32, tag="wconv", name="wconv")
            nc.sync.dma_start(out=wconv_sb[:], in_=w_conv[:])
            ident = const.tile([P, P], f32, tag="ident", name="ident")
            make_identity(nc, ident[:])
            idx_sb = []
            for m in range(MT):
                t = const.tile([P, 1], i32, tag=f"idx{m}", name=f"idx{m}")
                nc.sync.dma_start(out=t[:], in_=idx_d[m * P:(m + 1) * P, :])
                idx_sb.append(t)
            wout_sb = []
            for c in range(HC // P):
                t = const.tile([P, H], f32, tag=f"wout{c}", name=f"wout{c}")
                nc.sync.dma_start(out=t[:], in_=w_out_t[c * P:(c + 1) * P, :])
                wout_sb.append(t)

            # ---- body (idempotent; repeats>1 only for timing runs) ----
            for rep in range(repeats):
                R = f"r{rep}_"

                # bulk copy of the state pool shard, DRAM->DRAM on ACT ring
                NCHUNK = 8 if do_copy else 0
                rows = POOL // NCHUNK
                for ci in range(NCHUNK):
                    nc.scalar.dma_start(
                        out=state_out[ci * rows:(ci + 1) * rows, :],
                        in_=state_in[ci * rows:(ci + 1) * rows, :],
                    )

                # in_proj: proj[B, 768] = hs.T^T @ w_in_t, K=2048 streamed.
                # psum per m-tile: pa = proj[:, 0:512] (B_gate|C_gate), pb = x
                pa = [psum.tile([P, 512], f32, tag=f"pa{m}", name=f"{R}pa{m}")
                      for m in range(MT)]
                pb = [psum.tile([P, 256], f32, tag=f"pb{m}", name=f"{R}pb{m}")
                      for m in range(MT)]
                for k in range(KT if do_inproj else 0):
                    hs_k = wstream.tile([P, B], f32, tag="hsk", name=f"{R}hsk{k}")
                    nc.sync.dma_start(out=hs_k[:], in_=hs_t[k * P:(k + 1) * P, :])
                    w_k = wstream.tile([P, 3 * HC], f32, tag="wk", name=f"{R}wk{k}")
                    nc.sync.dma_start(out=w_k[:], in_=w_in_t[k * P:(k + 1) * P, :])
                    for m in range(MT):
                        lhsT = hs_k[:, m * P:(m + 1) * P]
                        nc.tensor.matmul(
                            pa[m][:], lhsT, w_k[:, 0:512],
                            start=(k == 0), stop=(k == KT - 1),
                        )
                        nc.tensor.matmul(
                            pb[m][:], lhsT, w_k[:, 512:768],
                            start=(k == 0), stop=(k == KT - 1),
                        )

                # gather current conv state rows for each request
                cur = []
                for m in range(MT):
                    t = sb.tile([P, 2 * HC], f32, tag=f"cur{m}", name=f"{R}cur{m}")
                    nc.gpsimd.indirect_dma_start(
                        out=t[:],
                        out_offset=None,
                        in_=state_in[:],
                        in_offset=bass.IndirectOffsetOnAxis(ap=idx_sb[m][:, :1], axis=0),
                    )
                    cur.append(t)

                # gating + depthwise conv (channelwise; request-partition)
                y_sb = []
                upd = []
                for m in range(MT):
                    x_sb = sb.tile([P, HC], f32, tag=f"x{m}", name=f"{R}x{m}")
                    nc.vector.tensor_copy(out=x_sb[:], in_=pb[m][:])
                    bx = sb.tile([P, HC], f32, tag=f"bx{m}", name=f"{R}bx{m}")
                    nc.vector.tensor_mul(out=bx[:], in0=pa[m][:, 0:HC], in1=x_sb[:])

                    cur_k = cur[m][:].rearrange("p (c k) -> p c k", k=2)
                    # conv_out = cur0*w0 + cur1*w1 + bx*w2
                    t0 = sb.tile([P, HC], f32, tag=f"t0{m}", name=f"{R}t0{m}")
                    nc.vector.tensor_mul(out=t0[:], in0=cur_k[:, :, 0],
                                         in1=wconv_sb[:, 0:HC])
                    t1 = sb.tile([P, HC], f32, tag=f"t1{m}", name=f"{R}t1{m}")
                    nc.vector.tensor_mul(out=t1[:], in0=cur_k[:, :, 1],
                                         in1=wconv_sb[:, HC:2 * HC])
                    t2 = sb.tile([P, HC], f32, tag=f"t2{m}", name=f"{R}t2{m}")
                    nc.vector.tensor_mul(out=t2[:], in0=bx[:],
                                         in1=wconv_sb[:, 2 * HC:3 * HC])
                    nc.vector.tensor_add(out=t0[:], in0=t0[:], in1=t1[:])
                    nc.vector.tensor_add(out=t0[:], in0=t0[:], in1=t2[:])
                    # y = C_gate * conv_out
                    y = sb.tile([P, HC], f32, tag=f"y{m}", name=f"{R}y{m}")
                    nc.vector.tensor_mul(out=y[:], in0=pa[m][:, HC:2 * HC], in1=t0[:])
                    y_sb.append(y)

                    # updated state rows: [cur[:,:,1], bx] interleaved as (c, k)
                    u = sb.tile([P, 2 * HC], f32, tag=f"upd{m}", name=f"{R}upd{m}")
                    u_k = u[:].rearrange("p (c k) -> p c k", k=2)
                    nc.vector.tensor_copy(out=u_k[:, :, 0], in_=cur_k[:, :, 1])
                    nc.vector.tensor_copy(out=u_k[:, :, 1], in_=bx[:])
                    upd.append(u)

                # scatter updated rows into the output state pool
                for m in range(MT):
                    nc.gpsimd.indirect_dma_start(
                        out=state_out[:],
                        out_offset=bass.IndirectOffsetOnAxis(ap=idx_sb[m][:, :1], axis=0),
                        in_=upd[m][:],
                        in_offset=None,
                    )

                # transpose y to channel-partition layout for out_proj
                yt_sb = []
                for c in range(HC // P):
                    pt = psum2.tile([P, B], f32, tag="pt", name=f"{R}pt{c}")
                    for m in range(MT):
                        nc.tensor.transpose(
                            out=pt[:, m * P:(m + 1) * P],
                            in_=y_sb[m][:, c * P:(c + 1) * P],
                            identity=ident[:],
                        )
                    t = sb.tile([P, B], f32, tag=f"yt{c}", name=f"{R}yt{c}")
                    nc.vector.tensor_copy(out=t[:], in_=pt[:])
                    yt_sb.append(t)

                # out_proj partial: out[B, 2048] = y^T.T @ w_out_t
                NO = H // 512  # 4 chunks of 512
                for m in range(MT):
                    for n in range(NO):
                        po = psum2.tile([P, 512], f32, tag="po", name=f"{R}po{m}_{n}")
                        for c in range(HC // P):
                            nc.tensor.matmul(
                                po[:],
                                yt_sb[c][:, m * P:(m + 1) * P],
                                wout_sb[c][:, n * 512:(n + 1) * 512],
                                start=(c == 0), stop=(c == HC // P - 1),
                            )
                        ot = osb_pool.tile([P, 512], f32, tag="osb", name=f"{R}osb{m}_{n}")
                        nc.vector.tensor_copy(out=ot[:], in_=po[:])
                        nc.sync.dma_start(
                            out=out_p[m * P:(m + 1) * P, n * 512:(n + 1) * 512],
                            in_=ot[:],
                        )

    nc.compile()
    return nc


def _get_nc(repeats=1):
    key = f"nc{repeats}"
    if key not in _CACHE:
        _CACHE[key] = _build(repeats)
    return _CACHE[key]


def make_in_maps(hidden_states, conv_state, req_pool_indices, W_in, W_out, conv_w):
    hs_t = np.ascontiguousarray(hidden_states.T.astype(np.float32, copy=False))
    idx = np.ascontiguousarray(req_pool_indices.astype(np.int32).reshape(B, 1))
    in_maps = []
    for c in range(N_CORES):
        lo, hi = c * HC, (c + 1) * HC
        w_in_rows = np.concatenate(
            [W_in[lo:hi, :], W_in[H + lo:H + hi, :], W_in[2 * H + lo:2 * H + hi, :]],
            axis=0,
        )  # [768, 2048]
        w_in_t = np.ascontiguousarray(w_in_rows.T)  # [2048, 768]
        w_out_t = np.ascontiguousarray(W_out[:, lo:hi].T)  # [256, 2048]
        wc = conv_w[lo:hi, :]  # [256, 3]
        wline = np.concatenate([wc[:, 0], wc[:, 1], wc[:, 2]])  # [768]
        w_conv_b = np.ascontiguousarray(np.broadcast_to(wline[None, :], (P, 3 * HC)))
        state_c = np.ascontiguousarray(conv_state[:, lo:hi, :]).reshape(POOL, 2 * HC)
        in_maps.append({
            "hs_t": hs_t,
            "w_in_t": w_in_t.astype(np.float32, copy=False),
            "w_out_t": w_out_t.astype(np.float32, copy=False),
            "w_conv": w_conv_b.astype(np.float32, copy=False),
            "state_in": state_c.astype(np.float32, copy=False),
            "idx": idx,
        })
    return in_maps


def unshard(results):
    out = np.zeros((B, H), dtype=np.float64)
    for r in results:
        out += r["out_p"].astype(np.float64)
    out = out.astype(np.float32)
    state = np.stack(
        [r["state_out"].reshape(POOL, HC, 2) for r in results], axis=1
    ).reshape(POOL, H, 2)
    return out, state


def run_sharded(inputs, trace=False, trace_cores=None):
    from concourse import bass_utils
    nc = _get_nc()
    in_maps = make_in_maps(**inputs)
    br = bass_utils.run_bass_kernel_spmd(
        nc, in_maps, list(range(N_CORES)), trace=trace, trace_cores=trace_cores,
    )
    return br


def kernel(hidden_states, conv_state, req_pool_indices, W_in, W_out, conv_w):
    br = run_sharded(dict(
        hidden_states=hidden_states, conv_state=conv_state,
        req_pool_indices=req_pool_indices, W_in=W_in, W_out=W_out, conv_w=conv_w,
    ))
    return unshard(br.results)


# revision 11
# speedup vs baseline: 6.4717x; 1.1086x over previous
"""Lfm2ShortConv decode-step kernel for 8x TRN2 NeuronCores (Bass/Tile).

Sharding: tensor-parallel over the channel dim H=2048 -> 256 channels/core.
 - in_proj: each core computes proj[:, ch] for its 3x256 W_in rows (weights
   read exactly once across cores).
 - gated depthwise conv + state gather/scatter: channelwise independent; each
   core owns a contiguous [8192, 256, 2] shard of the conv-state pool,
   bulk-copies it input->output (DRAM->DRAM) and indirect-scatters the 256
   updated rows.
 - out_proj: contraction over H is split; each core emits a partial [256,2048]
   output, summed on host (the unshard step).
"""

import numpy as np
from contextlib import ExitStack

B = 256          # batch / requests
H = 2048         # hidden channels
HC = H // 8      # channels per core = 256
POOL = 8192      # request pool slots
P = 128          # partitions
KT = H // P      # contraction tiles for in_proj = 16
N_CORES = 8

_CACHE = {}


def _build(repeats=1, do_copy=True, do_inproj=True, do_conv=True, do_outproj=True):
    import concourse.bass as bass
    import concourse.tile as tile
    import concourse.mybir as mybir
    from concourse import bacc
    from concourse.masks import make_identity

    f32 = mybir.dt.float32
    i32 = mybir.dt.int32

    nc = bacc.Bacc("TRN2", target_bir_lowering=False, debug=False)

    hs_t = nc.dram_tensor("hs_t", [H, B], f32, kind="ExternalInput").ap()
    w_in_t = nc.dram_tensor("w_in_t", [H, 3 * HC], f32, kind="ExternalInput").ap()
    w_out_t = nc.dram_tensor("w_out_t", [HC, H], f32, kind="ExternalInput").ap()
    w_conv = nc.dram_tensor("w_conv", [P, 3 * HC], f32, kind="ExternalInput").ap()
    state_in = nc.dram_tensor("state_in", [POOL, 2 * HC], f32, kind="ExternalInput").ap()
    idx_d = nc.dram_tensor("idx", [B, 1], i32, kind="ExternalInput").ap()

    out_p = nc.dram_tensor("out_p", [B, H], f32, kind="ExternalOutput").ap()
    state_out = nc.dram_tensor("state_out", [POOL, 2 * HC], f32, kind="ExternalOutput").ap()

    MT = B // P  # 2 request tiles

    with tile.TileContext(nc) as tc:
        with ExitStack() as ctx:
            const = ctx.enter_context(tc.tile_pool(name="const", bufs=1))
            wstream = ctx.enter_context(tc.tile_pool(name="wstream", bufs=3))
            sb = ctx.enter_context(tc.tile_pool(name="sb", bufs=1))
            osb_pool = ctx.enter_context(tc.tile_pool(name="osb", bufs=3))
            psum = ctx.enter_context(tc.tile_pool(name="psum", bufs=1, space="PSUM"))
            psum2 = ctx.enter_context(tc.tile_pool(name="psum2", bufs=2, space="PSUM"))

            # ---- constants (loaded once) ----
            wconv_sb = const.tile([P, 3 * HC], f32, tag="wconv", name="wconv")
            nc.sync.dma_start(out=wconv_sb[:], in_=w_conv[:])
            ident = const.tile([P, P], f32, tag="ident", name="ident")
            make_identity(nc, ident[:])
            idx_sb = []
            for m in range(MT):
                t = const.tile([P, 1], i32, tag=f"idx{m}", name=f"idx{m}")
                nc.sync.dma_start(out=t[:], in_=idx_d[m * P:(m + 1) * P, :])
                idx_sb.append(t)
            wout_sb = []
            for c in range(HC // P):
                t = const.tile([P, H], f32, tag=f"wout{c}", name=f"wout{c}")
                nc.sync.dma_start(out=t[:], in_=w_out_t[c * P:(c + 1) * P, :])
                wout_sb.append(t)

            # ---- body (idempotent; repeats>1 only for timing runs) ----
            for rep in range(repeats):
                R = f"r{rep}_"

                # gather current conv state rows first (front of the SWDGE ring)
                cur = []
                for m in range(MT):
                    t = sb.tile([P, 2 * HC], f32, tag=f"cur{m}", name=f"{R}cur{m}")
                    nc.gpsimd.indirect_dma_start(
                        out=t[:],
                        out_offset=None,
                        in_=state_in[:],
                        in_offset=bass.IndirectOffsetOnAxis(ap=idx_sb[m][:, :1], axis=0),
                    )
                    cur.append(t)

                # bulk copy of the state pool shard routed through SBUF:
                # HBM->SBUF loads on the ACT ring overlap SBUF->HBM stores on
                # the SP ring (direct DRAM->DRAM runs well below line rate).
                NCHUNK = 4 if do_copy else 0
                rows = POOL // max(NCHUNK, 1)          # 2048 rows = 4 MB
                cpw = rows * 2 * HC // P               # 8192 f32 per partition
                st_in_t = state_in.rearrange("(c p r) w -> c p (r w)",
                                             c=max(NCHUNK, 1), p=P)
                st_out_t = state_out.rearrange("(c p r) w -> c p (r w)",
                                               c=max(NCHUNK, 1), p=P)
                for ci in range(NCHUNK):
                    cp = sb.tile([P, cpw], f32, tag="cp", bufs=2, name=f"{R}cp{ci}")
                    nc.scalar.dma_start(out=cp[:], in_=st_in_t[ci])
                    nc.sync.dma_start(out=st_out_t[ci], in_=cp[:])

                # in_proj: proj[B, 768] = hs.T^T @ w_in_t, K=2048 streamed.
                # psum per m-tile: pa = proj[:, 0:512] (B_gate|C_gate), pb = x
                pa = [psum.tile([P, 512], f32, tag=f"pa{m}", name=f"{R}pa{m}")
                      for m in range(MT)]
                pb = [psum.tile([P, 256], f32, tag=f"pb{m}", name=f"{R}pb{m}")
                      for m in range(MT)]
                for k in range(KT if do_inproj else 0):
                    hs_k = wstream.tile([P, B], f32, tag="hsk", name=f"{R}hsk{k}")
                    nc.sync.dma_start(out=hs_k[:], in_=hs_t[k * P:(k + 1) * P, :])
                    w_k = wstream.tile([P, 3 * HC], f32, tag="wk", name=f"{R}wk{k}")
                    nc.sync.dma_start(out=w_k[:], in_=w_in_t[k * P:(k + 1) * P, :])
                    for m in range(MT):
                        lhsT = hs_k[:, m * P:(m + 1) * P]
                        nc.tensor.matmul(
                            pa[m][:], lhsT, w_k[:, 0:512],
                            start=(k == 0), stop=(k == KT - 1),
                        )
                        nc.tensor.matmul(
                            pb[m][:], lhsT, w_k[:, 512:768],
                            start=(k == 0), stop=(k == KT - 1),
                        )

                # gating + depthwise conv (channelwise; request-partition)
                y_sb = []
                upd = []
                for m in range(MT):
                    x_sb = sb.tile([P, HC], f32, tag=f"x{m}", name=f"{R}x{m}")
                    nc.vector.tensor_copy(out=x_sb[:], in_=pb[m][:])
                    bx = sb.tile([P, HC], f32, tag=f"bx{m}", name=f"{R}bx{m}")
                    nc.vector.tensor_mul(out=bx[:], in0=pa[m][:, 0:HC], in1=x_sb[:])

                    cur_k = cur[m][:].rearrange("p (c k) -> p c k", k=2)
                    # conv_out = cur0*w0 + cur1*w1 + bx*w2
                    t0 = sb.tile([P, HC], f32, tag=f"t0{m}", name=f"{R}t0{m}")
                    nc.vector.tensor_mul(out=t0[:], in0=cur_k[:, :, 0],
                                         in1=wconv_sb[:, 0:HC])
                    t1 = sb.tile([P, HC], f32, tag=f"t1{m}", name=f"{R}t1{m}")
                    nc.vector.tensor_mul(out=t1[:], in0=cur_k[:, :, 1],
                                         in1=wconv_sb[:, HC:2 * HC])
                    t2 = sb.tile([P, HC], f32, tag=f"t2{m}", name=f"{R}t2{m}")
                    nc.vector.tensor_mul(out=t2[:], in0=bx[:],
                                         in1=wconv_sb[:, 2 * HC:3 * HC])
                    nc.vector.tensor_add(out=t0[:], in0=t0[:], in1=t1[:])
                    nc.vector.tensor_add(out=t0[:], in0=t0[:], in1=t2[:])
                    # y = C_gate * conv_out
                    y = sb.tile([P, HC], f32, tag=f"y{m}", name=f"{R}y{m}")
                    nc.vector.tensor_mul(out=y[:], in0=pa[m][:, HC:2 * HC], in1=t0[:])
                    y_sb.append(y)

                    # updated state rows: [cur[:,:,1], bx] interleaved as (c, k)
                    u = sb.tile([P, 2 * HC], f32, tag=f"upd{m}", name=f"{R}upd{m}")
                    u_k = u[:].rearrange("p (c k) -> p c k", k=2)
                    nc.vector.tensor_copy(out=u_k[:, :, 0], in_=cur_k[:, :, 1])
                    nc.vector.tensor_copy(out=u_k[:, :, 1], in_=bx[:])
                    upd.append(u)

                # scatter updated rows into the output state pool
                for m in range(MT):
                    nc.gpsimd.indirect_dma_start(
                        out=state_out[:],
                        out_offset=bass.IndirectOffsetOnAxis(ap=idx_sb[m][:, :1], axis=0),
                        in_=upd[m][:],
                        in_offset=None,
                    )

                # transpose y to channel-partition layout for out_proj
                yt_sb = []
                for c in range(HC // P):
                    pt = psum2.tile([P, B], f32, tag="pt", name=f"{R}pt{c}")
                    for m in range(MT):
                        nc.tensor.transpose(
                            out=pt[:, m * P:(m + 1) * P],
                            in_=y_sb[m][:, c * P:(c + 1) * P],
                            identity=ident[:],
                        )
                    t = sb.tile([P, B], f32, tag=f"yt{c}", name=f"{R}yt{c}")
                    nc.vector.tensor_copy(out=t[:], in_=pt[:])
                    yt_sb.append(t)

                # out_proj partial: out[B, 2048] = y^T.T @ w_out_t
                NO = H // 512  # 4 chunks of 512
                for m in range(MT):
                    for n in range(NO):
                        po = psum2.tile([P, 512], f32, tag="po", name=f"{R}po{m}_{n}")
                        for c in range(HC // P):
                            nc.tensor.matmul(
                                po[:],
                                yt_sb[c][:, m * P:(m + 1) * P],
                                wout_sb[c][:, n * 512:(n + 1) * 512],
                                start=(c == 0), stop=(c == HC // P - 1),
                            )
                        ot = osb_pool.tile([P, 512], f32, tag="osb", name=f"{R}osb{m}_{n}")
                        nc.vector.tensor_copy(out=ot[:], in_=po[:])
                        nc.sync.dma_start(
                            out=out_p[m * P:(m + 1) * P, n * 512:(n + 1) * 512],
                            in_=ot[:],
                        )

    nc.compile()
    return nc


def _get_nc(repeats=1):
    key = f"nc{repeats}"
    if key not in _CACHE:
        _CACHE[key] = _build(repeats)
    return _CACHE[key]


def make_in_maps(hidden_states, conv_state, req_pool_indices, W_in, W_out, conv_w):
    hs_t = np.ascontiguousarray(hidden_states.T.astype(np.float32, copy=False))
    idx = np.ascontiguousarray(req_pool_indices.astype(np.int32).reshape(B, 1))
    in_maps = []
    for c in range(N_CORES):
        lo, hi = c * HC, (c + 1) * HC
        w_in_rows = np.concatenate(
            [W_in[lo:hi, :], W_in[H + lo:H + hi, :], W_in[2 * H + lo:2 * H + hi, :]],
            axis=0,
        )  # [768, 2048]
        w_in_t = np.ascontiguousarray(w_in_rows.T)  # [2048, 768]
        w_out_t = np.ascontiguousarray(W_out[:, lo:hi].T)  # [256, 2048]
        wc = conv_w[lo:hi, :]  # [256, 3]
        wline = np.concatenate([wc[:, 0], wc[:, 1], wc[:, 2]])  # [768]
        w_conv_b = np.ascontiguousarray(np.broadcast_to(wline[None, :], (P, 3 * HC)))
        state_c = np.ascontiguousarray(conv_state[:, lo:hi, :]).reshape(POOL, 2 * HC)
        in_maps.append({
            "hs_t": hs_t,
            "w_in_t": w_in_t.astype(np.float32, copy=False),
            "w_out_t": w_out_t.astype(np.float32, copy=False),
            "w_conv": w_conv_b.astype(np.float32, copy=False),
            "state_in": state_c.astype(np.float32, copy=False),
            "idx": idx,
        })
    return in_maps


def unshard(results):
    out = np.zeros((B, H), dtype=np.float64)
    for r in results:
        out += r["out_p"].astype(np.float64)
    out = out.astype(np.float32)
    state = np.stack(
        [r["state_out"].reshape(POOL, HC, 2) for r in results], axis=1
    ).reshape(POOL, H, 2)
    return out, state


def run_sharded(inputs, trace=False, trace_cores=None):
    from concourse import bass_utils
    nc = _get_nc()
    in_maps = make_in_maps(**inputs)
    br = bass_utils.run_bass_kernel_spmd(
        nc, in_maps, list(range(N_CORES)), trace=trace, trace_cores=trace_cores,
    )
    return br


def kernel(hidden_states, conv_state, req_pool_indices, W_in, W_out, conv_w):
    br = run_sharded(dict(
        hidden_states=hidden_states, conv_state=conv_state,
        req_pool_indices=req_pool_indices, W_in=W_in, W_out=W_out, conv_w=conv_w,
    ))
    return unshard(br.results)
